# revision 5
# baseline (speedup 1.0000x reference)
"""MACCL loss kernel for Trainium2 (8 NeuronCores, SPMD data-parallel).

Strategy (v2: fp8 DoubleRow)
----------------------------
The O(B^2 D) contrastive part dominates (B=8192, D=256).  We permute the
batch so label-0 rows come first (split point n0 is baked into the
program at build time), shard rows 1024-per-core, and on each core:

  prologue (streamed per 128-row tile, chunked, overlapped with main):
    - DMA raw feature tiles [128, 256] fp32 (batched 4 tiles per DMA)
    - row norms^2 via DVE scalar_tensor_tensor accum (fp32, exact -> the
      center loss keeps full precision)
    - rcp' = 16/|f| via ACT Ln + Exp(scale=-0.5, bias=ln16)
    - DVE tensor_scalar (per-partition AP scalar) raw*rcp' -> bf16
    - DMA XBAR transpose bf16 [128,256] -> [128,2,128] staging (the K-dim
      mapping d <-> (partition, ktile) is whatever the XBAR produces; it
      only needs to be consistent between both matmul operands)
    - Pool tensor_copy bf16 -> fp8e4 into the resident operand buffers

  main loop (g-outer over 2048-wide column groups, m-inner over the
  core's eight 128-row tiles):
    - one PE DoubleRow fp8 matmul per 512-col window: lhsT [128,2,128],
      rhs [128,2,512] (K=256 in a single pass, 2 fp8 mults/cell/cycle)
    - ACT exp(psum * 1/(256*T)) in place with accum_out giving per-row
      sums per label segment (columns are label-sorted)
    - DVE reduces the few segment partials into S0/S1

  f-hat is pre-scaled by 16 (exact power of 2) before the fp8 cast so
  elements ~N(0,1) use e4m3's subnormal range better; the exp scale
  1/(256*T) compensates exactly.

  outputs per core: stats [128, 40] fp32 = {norms^2, rowsum, S0, S1,
  exp(diag)} for its 1024 rows.  Host does the O(B) finalization.

The diagonal (self-similarity) term is computed on-device from the same
fp8 operands the main matmul consumes (same DoubleRow mode, same 512-col
window offset), so the host-side pos_sum = S_same - d subtraction
cancels bitwise.
"""

import os
import sys

for _p in ("/root/.axon_site", "/root/.axon_site/_ro/trn_rl_repo",
           "/root/.axon_site/_ro/pypackages", "/opt/trn_rl_repo", "/opt/pypackages"):
    if os.path.isdir(_p) and _p not in sys.path:
        sys.path.append(_p)

import math
import numpy as np
from contextlib import ExitStack

import concourse.bass as bass
import concourse.bacc as bacc
import concourse.tile as tile
from concourse import mybir
from concourse.bass_utils import run_bass_kernel_spmd

F32 = mybir.dt.float32
BF16 = mybir.dt.bfloat16
F8 = mybir.dt.float8e4

P = 128
D = 256
NCORES = 8
TEMPERATURE = 0.07
MARGIN_BASE = 0.5
LAMBDA_SIGMA = 0.3
LAMBDA_RESOLUTION = 0.3
RESOLUTION_RATIO = 224.0 / 900.0
ALPHA, BETA, GAMMA = 1.0, 1.0, 0.5

FSCALE = 16.0                      # exact power-of-2 pre-scale before fp8
EXP_SCALE = 1.0 / (FSCALE * FSCALE * TEMPERATURE)

# engine assignment knobs for quick A/B on hardware
STT_ENGINE = os.environ.get("MACCL_STT", "vector")      # norms^2 accum
SCALE_ENGINE = os.environ.get("MACCL_SCALE", "vector")  # raw*rcp -> bf16
CAST_ENGINE = os.environ.get("MACCL_CAST", "gpsimd")    # bf16 -> fp8 copy
DMA_BATCH = int(os.environ.get("MACCL_DMA_BATCH", "4"))


def _segment_ranges(B, n0, gw):
    """Column ranges per gw-wide group, split at the label boundary n0.

    Returns (ranges, k0): ranges = [(g, start, end, label)...] in ascending
    column order (so all label-0 ranges come first), k0 = #label-0 ranges.
    """
    ranges = []
    ng = B // gw
    for g in range(ng):
        lo, hi = g * gw, (g + 1) * gw
        cuts = sorted({lo, hi, min(max(n0, lo), hi)})
        for s, e in zip(cuts, cuts[1:]):
            if e > s:
                ranges.append((g, s, e, 0 if e <= n0 else 1))
    k0 = sum(1 for r in ranges if r[3] == 0)
    return ranges, k0


def build_program(n0, B=8192, bpc=1024):
    """Build the SPMD Bass program (one NeuronCore's view)."""
    gw = 2048
    ng = B // gw
    nsub = gw // 512
    nt_mine = bpc // P
    mrow = bpc // P

    ranges, k0 = _segment_ranges(B, n0, gw)
    nslots = len(ranges)
    k1 = nslots - k0

    nc = bacc.Bacc("TRN2", target_bir_lowering=False, debug=False,
                   num_devices=NCORES)
    feat_all = nc.dram_tensor("feat_all", [B, D], F32, kind="ExternalInput").ap()
    feat_mine = nc.dram_tensor("feat_mine", [bpc, D], F32, kind="ExternalInput").ap()
    ident_d = nc.dram_tensor("ident", [P, P], F32, kind="ExternalInput").ap()
    stats_d = nc.dram_tensor("stats", [P, 5 * mrow], F32, kind="ExternalOutput").ap()

    fa_r = feat_all.rearrange("(n p) d -> n p d", p=P)
    fm_r = feat_mine.rearrange("(n p) d -> n p d", p=P)

    AX = mybir.AxisListType.X
    MUL = mybir.AluOpType.mult
    AF = mybir.ActivationFunctionType
    DR = mybir.MatmulPerfMode.DoubleRow
    LN16 = math.log(FSCALE)

    with tile.TileContext(nc) as tc, ExitStack() as ctx:
        singles = ctx.enter_context(tc.tile_pool(name="singles", bufs=1))
        raw_pool = ctx.enter_context(tc.tile_pool(name="raw", bufs=6))
        b16_pool = ctx.enter_context(tc.tile_pool(name="b16", bufs=8))
        t16_pool = ctx.enter_context(tc.tile_pool(name="t16", bufs=8))
        scr_pool = ctx.enter_context(tc.tile_pool(name="scr", bufs=2))
        small = ctx.enter_context(tc.tile_pool(name="small", bufs=3))
        acc_pool = ctx.enter_context(tc.tile_pool(name="acc", bufs=mrow))
        ps_pool = ctx.enter_context(tc.tile_pool(name="ps", bufs=2, space="PSUM"))

        ident_t = singles.tile([P, P], F32)
        nc.sync.dma_start(ident_t, ident_d)
        allT = singles.tile([P, 2, B], F8)
        mineT = singles.tile([P, 2, bpc], F8)
        stats_sb = singles.tile([P, 5 * mrow], F32)

        stt_eng = getattr(nc, STT_ENGINE)
        scale_eng = getattr(nc, SCALE_ENGINE)
        cast_eng = getattr(nc, CAST_ENGINE)

        def prologue_chunk(srcs_r, t0, n, destT, col0, mine_base):
            """Process a chunk of n row tiles: stats, normalize, transpose.

            srcs_r: rearranged DRAM view [nt, 128, 256].  t0: first tile
            index in that view.  destT: allT or mineT.  col0: first dest
            column-tile index.  mine_base: row-tile index of tile t0 within
            the core's own block, or None.
            """
            nrm2c = small.tile([P, n], F32, tag="nrm2c")
            raws = []
            bi = 0
            while bi < n:
                bn = min(DMA_BATCH, n - bi)
                rawb = raw_pool.tile([P, bn, D], F32, tag="raw")
                src = srcs_r[t0 + bi:t0 + bi + bn, :, :].rearrange("n p d -> p n d")
                nc.sync.dma_start(rawb, src)
                for i in range(bn):
                    raws.append(rawb[:, i, :])
                bi += bn
            for i, raw in enumerate(raws):
                scr = scr_pool.tile([P, D], F32, tag="scr")
                stt_eng.scalar_tensor_tensor(
                    out=scr, in0=raw, scalar=1.0, in1=raw,
                    op0=MUL, op1=MUL, accum_out=nrm2c[:, i:i + 1])
                if mine_base is not None:
                    mi = mine_base + i
                    nc.vector.reduce_sum(stats_sb[:, mrow + mi:mrow + mi + 1],
                                         raw, axis=AX)
            if mine_base is not None:
                # norms^2 for own rows (before clamping; values >> clamp).
                nc.vector.tensor_copy(out=stats_sb[:, 0:n], in_=nrm2c[:, 0:n])
            nc.vector.tensor_scalar_max(nrm2c, nrm2c, 1e-24)
            lnc = small.tile([P, n], F32, tag="lnc")
            # ln(n2/256); the Exp(scale=-0.5) then yields 16/|f| exactly
            nc.scalar.activation(lnc, nrm2c, AF.Ln, scale=1.0 / (FSCALE * FSCALE))
            rcpc = small.tile([P, n], F32, tag="rcpc")
            nc.scalar.activation(rcpc, lnc, AF.Exp, scale=-0.5)
            for i, raw in enumerate(raws):
                b16 = b16_pool.tile([P, D], BF16, tag="b16")
                scale_eng.tensor_scalar_mul(b16, raw, rcpc[:, i:i + 1])
                t16 = t16_pool.tile([P, 2, P], BF16, tag="t16")
                nc.sync.dma_start_transpose(t16, b16)
                col = (col0 + i) * P
                cast_eng.tensor_copy(out=destT[:, :, col:col + P], in_=t16)

        # ---- own rows first (mineT + per-row stats) ----
        prologue_chunk(fm_r, 0, nt_mine, mineT, 0, 0)

        # ---- exact diagonal terms ----
        # psd = mineT_m^T @ mineT_m reproduces, bit-for-bit, the diagonal
        # elements the big matmul produces (same DoubleRow datapath, same
        # 512-col window offset); exp through the same ACT path then a
        # masked row-reduce against the identity extracts e_ii, so the
        # host-side pos_sum = S_same - d subtraction cancels exactly.
        W = 512
        for m in range(mrow):
            psd = ps_pool.tile([P, gw], F32, tag="ps", name=f"psd{m}")
            c0 = (m * P // W) * W              # W-col group holding block m
            off = m * P - c0                   # block-local diag offset
            nc.tensor.matmul(psd[:, 0:W], mineT[:, :, m * P:(m + 1) * P],
                             mineT[:, :, c0:c0 + W], start=True, stop=True,
                             perf_mode=DR)
            nc.scalar.activation(psd[:, off:off + P], psd[:, off:off + P],
                                 AF.Exp, scale=EXP_SCALE)
            scrd = scr_pool.tile([P, D], F32, tag="scr", name=f"scrd{m}")
            nc.vector.scalar_tensor_tensor(
                out=scrd[:, 0:P], in0=psd[:, off:off + P], scalar=1.0,
                in1=ident_t, op0=MUL, op1=MUL,
                accum_out=stats_sb[:, 4 * mrow + m:4 * mrow + m + 1])

        accs = [acc_pool.tile([P, nslots], F32, tag="acc", name=f"acc{m}")
                for m in range(mrow)]

        # ---- interleaved: column-chunk prologue + that group's matmuls ----
        tiles_per_g = gw // P
        for g in range(ng):
            t0 = g * tiles_per_g
            prologue_chunk(fa_r, t0, tiles_per_g, allT, t0, None)
            for m in range(mrow):
                psg = ps_pool.tile([P, gw], F32, tag="ps")
                lhsT = mineT[:, :, m * P:(m + 1) * P]
                for sub in range(nsub):
                    ncol = (g * nsub + sub) * 512
                    nc.tensor.matmul(
                        psg[:, sub * 512:(sub + 1) * 512], lhsT,
                        allT[:, :, ncol:ncol + 512],
                        start=True, stop=True, perf_mode=DR)
                for slot, (gg, s, e, _lab) in enumerate(ranges):
                    if gg != g:
                        continue
                    rs, re = s - g * gw, e - g * gw
                    nc.scalar.activation(
                        psg[:, rs:re], psg[:, rs:re], AF.Exp,
                        scale=EXP_SCALE,
                        accum_out=accs[m][:, slot:slot + 1])

        # ---- per-row-tile S0/S1 ----
        for m in range(mrow):
            s0 = stats_sb[:, 2 * mrow + m:2 * mrow + m + 1]
            s1 = stats_sb[:, 3 * mrow + m:3 * mrow + m + 1]
            if k0 > 0:
                nc.vector.reduce_sum(s0, accs[m][:, 0:k0], axis=AX)
            else:
                nc.vector.memset(s0, 0.0)
            if k1 > 0:
                nc.vector.reduce_sum(s1, accs[m][:, k0:nslots], axis=AX)
            else:
                nc.vector.memset(s1, 0.0)

        nc.sync.dma_start(stats_d, stats_sb)

    nc.compile()
    return nc


_PROGRAM_CACHE = {}


def _get_program(n0):
    key = (n0, STT_ENGINE, SCALE_ENGINE, CAST_ENGINE, DMA_BATCH)
    if key not in _PROGRAM_CACHE:
        _PROGRAM_CACHE[key] = build_program(n0)
    return _PROGRAM_CACHE[key]


def run_device(features, labels, trace=False):
    """Run the Bass kernel on 8 cores.  Returns (per-row device stats dict
    aligned to the label-sorted permutation, permutation order, n0, raw
    BassKernelResults)."""
    B, d = features.shape
    assert d == D and B % NCORES == 0
    bpc = B // NCORES
    mrow = bpc // P

    order = np.argsort(labels, kind="stable")
    n0 = int((labels == 0).sum())
    fp = np.ascontiguousarray(features[order]).astype(np.float32, copy=False)

    nc = _get_program(n0)
    ident = np.eye(P, dtype=np.float32)
    in_maps = [
        {"feat_all": fp,
         "feat_mine": np.ascontiguousarray(fp[c * bpc:(c + 1) * bpc]),
         "ident": ident}
        for c in range(NCORES)
    ]
    res = run_bass_kernel_spmd(nc, in_maps, list(range(NCORES)), trace=trace)

    parts = []
    for c in range(NCORES):
        st = res.results[c]["stats"]          # [128, 5*mrow]
        arr = st.reshape(P, 5, mrow).transpose(1, 2, 0).reshape(5, bpc)
        parts.append(arr)
    full = np.concatenate(parts, axis=1)      # [5, B] in permuted row order
    stats = {"norms2": full[0], "rowsum": full[1], "S0": full[2],
             "S1": full[3], "d": full[4]}
    return stats, order, n0, res


def finalize(stats, order, n0, labels, normal_center, running_sigma, B):
    """Host O(B) finalization mirroring the reference formulas (float64)."""
    labels_p = labels[order]
    nmf = (labels_p == 0)
    amf = (labels_p == 1)
    norms2 = stats["norms2"].astype(np.float64)
    rowsum = stats["rowsum"].astype(np.float64)
    S0 = stats["S0"].astype(np.float64)
    S1 = stats["S1"].astype(np.float64)
    ddiag = stats["d"].astype(np.float64)

    dist_sq = norms2  # center == 0
    n_normal = float(nmf.sum())

    with np.errstate(divide="ignore", invalid="ignore"):
        n_el = n_normal * D
        masked_sum = float((rowsum * nmf).sum())
        mean = masked_sum / n_el
        sum_sq_m = float((norms2 * nmf).sum())
        var = (sum_sq_m - 2.0 * mean * masked_sum + mean * mean * n_el) / (n_el - 1.0)
        sigma_new = 0.9 * float(running_sigma) + 0.1 * np.sqrt(var)

        m_adaptive = (MARGIN_BASE + LAMBDA_SIGMA * sigma_new
                      + LAMBDA_RESOLUTION * (1.0 - RESOLUTION_RATIO))
        dist = np.sqrt(np.maximum(dist_sq, 0.0))
        r_center = dist_sq * nmf
        r_margin = np.maximum(m_adaptive - dist, 0.0) * amf

        S_same = np.where(nmf, S0, S1)
        S_diff = np.where(nmf, S1, S0)
        pos_sum = S_same - ddiag
        neg_sum = S_diff
        n1 = B - n0
        cnt_pos = np.where(nmf, n0 - 1, n1 - 1)
        cnt_neg = np.where(nmf, n1, n0)
        has_both = (cnt_pos > 0) & (cnt_neg > 0)
        pos_safe = np.where(has_both, np.maximum(pos_sum, 1e-12), 1.0)
        den_safe = np.where(has_both, pos_sum + neg_sum + 1e-8, 1.0)
        r_con = np.where(has_both, -np.log(pos_safe / den_safe), 0.0)

        raw_total = ALPHA * r_center + BETA * r_margin + GAMMA * r_con
        total = raw_total.mean()
    return np.array(total, dtype=np.float32)


def _finalize_general_center(stats, order, n0, labels, normal_center,
                             running_sigma, B, features):
    """Fallback for a nonzero normal_center (not used for spec inputs)."""
    labels_p = labels[order]
    fp = features[order].astype(np.float64)
    c = np.asarray(normal_center, dtype=np.float64)
    qc = fp @ c
    norms2 = stats["norms2"].astype(np.float64)
    dist_sq = norms2 - 2.0 * qc + float((c * c).sum())
    nmf = (labels_p == 0)
    amf = (labels_p == 1)
    rowsum = stats["rowsum"].astype(np.float64)
    S0 = stats["S0"].astype(np.float64)
    S1 = stats["S1"].astype(np.float64)
    ddiag = stats["d"].astype(np.float64)
    n_normal = float(nmf.sum())
    with np.errstate(divide="ignore", invalid="ignore"):
        n_el = n_normal * D
        masked_sum = float((rowsum * nmf).sum())
        mean = masked_sum / n_el
        sum_sq_m = float((norms2 * nmf).sum())
        var = (sum_sq_m - 2.0 * mean * masked_sum + mean * mean * n_el) / (n_el - 1.0)
        sigma_new = 0.9 * float(running_sigma) + 0.1 * np.sqrt(var)
        m_adaptive = (MARGIN_BASE + LAMBDA_SIGMA * sigma_new
                      + LAMBDA_RESOLUTION * (1.0 - RESOLUTION_RATIO))
        dist = np.sqrt(np.maximum(dist_sq, 0.0))
        r_center = dist_sq * nmf
        r_margin = np.maximum(m_adaptive - dist, 0.0) * amf
        S_same = np.where(nmf, S0, S1)
        S_diff = np.where(nmf, S1, S0)
        pos_sum = S_same - ddiag
        neg_sum = S_diff
        n1 = B - n0
        cnt_pos = np.where(nmf, n0 - 1, n1 - 1)
        cnt_neg = np.where(nmf, n1, n0)
        has_both = (cnt_pos > 0) & (cnt_neg > 0)
        pos_safe = np.where(has_both, np.maximum(pos_sum, 1e-12), 1.0)
        den_safe = np.where(has_both, pos_sum + neg_sum + 1e-8, 1.0)
        r_con = np.where(has_both, -np.log(pos_safe / den_safe), 0.0)
        total = (ALPHA * r_center + BETA * r_margin + GAMMA * r_con).mean()
    return np.array(total, dtype=np.float32)


def kernel(features, labels, normal_center, running_sigma):
    features = np.asarray(features, dtype=np.float32)
    labels = np.asarray(labels, dtype=np.int32)
    normal_center = np.asarray(normal_center, dtype=np.float32)
    running_sigma = np.float32(np.asarray(running_sigma))
    B = features.shape[0]

    stats, order, n0, _res = run_device(features, labels)
    if float((np.asarray(normal_center, np.float64) ** 2).sum()) != 0.0:
        return _finalize_general_center(stats, order, n0, labels,
                                        normal_center, running_sigma, B,
                                        features)
    return finalize(stats, order, n0, labels, normal_center, running_sigma, B)


# revision 6
# speedup vs baseline: 1.4920x; 1.4920x over previous
"""MACCL loss kernel for Trainium2 (8 NeuronCores, SPMD data-parallel).

Strategy (v3: fp8 DoubleRow + chunked XBAR transposes)
------------------------------------------------------
The O(B^2 D) contrastive part dominates (B=8192, D=256).  We permute the
batch so label-0 rows come first (split point n0 baked into the program),
shard rows 1024-per-core, and on each core:

  prologue per 16-row-tile chunk (overlapped with the main loop):
    - 4x DMA raw feature tiles [128, 4, 256] fp32 into one chunk tile
    - ONE DVE scalar_tensor_tensor: bf16 chunk = raw * rcp (per-row scale
      broadcast via a stride-0 AP; rcp = 16/|f| is host-computed and fed
      as a tiny input -- it is only a normalization helper, all loss
      statistics stay on-device)
    - DMA chunk to a DRAM staging tile (restacks [128,16,256] SBUF ->
      [2048, 256] DRAM rows)
    - ONE XBAR dma_start_transpose DRAM -> SBUF [128, 2, 2048] bf16
      (the K-dim mapping d <-> (partition, ktile) is whatever the XBAR
      produces; it only needs to be consistent across operands)
    - ONE DVE tensor_copy cast bf16 -> fp8e4 into the resident operands
    - (mine chunk only) per-tile norms^2 (exact fp32 via STT accum) and
      row sums for the center/sigma statistics

  main loop (g-outer over 2048-wide column groups, m-inner over the
  core's eight 128-row tiles):
    - one PE DoubleRow fp8 matmul per 512-col window: lhsT [128,2,128],
      rhs [128,2,512] (K=256 in a single pass, 2 fp8 mults/cell/cycle)
    - ACT exp(psum * 1/(256*T)) in place with accum_out giving per-row
      sums per label segment (columns are label-sorted)
    - DVE reduces the few segment partials into S0/S1

  f-hat is pre-scaled by 16 (exact power of 2) before the fp8 cast so
  elements ~N(0,1) use e4m3's range better; the exp scale 1/(256*T)
  compensates exactly.

  outputs per core: stats [128, 40] fp32 = {norms^2, rowsum, S0, S1,
  exp(diag)} for its 1024 rows.  Host does the O(B) finalization.

The diagonal (self-similarity) term is computed on-device from the same
fp8 operands the main matmul consumes (same DoubleRow mode, same 512-col
window offset), so the host-side pos_sum = S_same - d subtraction
cancels bitwise.
"""

import os
import sys

for _p in ("/root/.axon_site", "/root/.axon_site/_ro/trn_rl_repo",
           "/root/.axon_site/_ro/pypackages", "/opt/trn_rl_repo", "/opt/pypackages"):
    if os.path.isdir(_p) and _p not in sys.path:
        sys.path.append(_p)

import numpy as np
from contextlib import ExitStack

import concourse.bass as bass
import concourse.bacc as bacc
import concourse.tile as tile
from concourse import mybir
from concourse.bass_utils import run_bass_kernel_spmd

F32 = mybir.dt.float32
BF16 = mybir.dt.bfloat16
F8 = mybir.dt.float8e4

P = 128
D = 256
NCORES = 8
TEMPERATURE = 0.07
MARGIN_BASE = 0.5
LAMBDA_SIGMA = 0.3
LAMBDA_RESOLUTION = 0.3
RESOLUTION_RATIO = 224.0 / 900.0
ALPHA, BETA, GAMMA = 1.0, 1.0, 0.5

FSCALE = 16.0                      # exact power-of-2 pre-scale before fp8
EXP_SCALE = 1.0 / (FSCALE * FSCALE * TEMPERATURE)

DMA_BATCH = int(os.environ.get("MACCL_DMA_BATCH", "4"))


def _segment_ranges(B, n0, gw):
    """Column ranges per gw-wide group, split at the label boundary n0."""
    ranges = []
    ng = B // gw
    for g in range(ng):
        lo, hi = g * gw, (g + 1) * gw
        cuts = sorted({lo, hi, min(max(n0, lo), hi)})
        for s, e in zip(cuts, cuts[1:]):
            if e > s:
                ranges.append((g, s, e, 0 if e <= n0 else 1))
    k0 = sum(1 for r in ranges if r[3] == 0)
    return ranges, k0


def build_program(n0, B=8192, bpc=1024):
    """Build the SPMD Bass program (one NeuronCore's view)."""
    gw = 2048
    ng = B // gw
    nsub = gw // 512
    nt_mine = bpc // P
    nt_all = B // P
    mrow = bpc // P

    ranges, k0 = _segment_ranges(B, n0, gw)
    nslots = len(ranges)
    k1 = nslots - k0

    nc = bacc.Bacc("TRN2", target_bir_lowering=False, debug=False,
                   num_devices=NCORES)
    feat_all = nc.dram_tensor("feat_all", [B, D], F32, kind="ExternalInput").ap()
    feat_mine = nc.dram_tensor("feat_mine", [bpc, D], F32, kind="ExternalInput").ap()
    rcp_all_d = nc.dram_tensor("rcp_all", [nt_all, P], F32, kind="ExternalInput").ap()
    rcp_mine_d = nc.dram_tensor("rcp_mine", [nt_mine, P], F32, kind="ExternalInput").ap()
    ident_d = nc.dram_tensor("ident", [P, P], F32, kind="ExternalInput").ap()
    stats_d = nc.dram_tensor("stats", [P, 5 * mrow], F32, kind="ExternalOutput").ap()

    fa_r = feat_all.rearrange("(n p) d -> n p d", p=P)
    fm_r = feat_mine.rearrange("(n p) d -> n p d", p=P)

    AX = mybir.AxisListType.X
    MUL = mybir.AluOpType.mult
    AF = mybir.ActivationFunctionType
    DR = mybir.MatmulPerfMode.DoubleRow

    with tile.TileContext(nc) as tc, ExitStack() as ctx:
        singles = ctx.enter_context(tc.tile_pool(name="singles", bufs=1))
        raw_pool = ctx.enter_context(tc.tile_pool(name="raw", bufs=2))
        b16_pool = ctx.enter_context(tc.tile_pool(name="b16", bufs=2))
        c16_pool = ctx.enter_context(tc.tile_pool(name="c16", bufs=2))
        stage_pool = ctx.enter_context(
            tc.tile_pool(name="stage", bufs=2, space="DRAM"))
        scr_pool = ctx.enter_context(tc.tile_pool(name="scr", bufs=2))
        acc_pool = ctx.enter_context(tc.tile_pool(name="acc", bufs=mrow))
        ps_pool = ctx.enter_context(tc.tile_pool(name="ps", bufs=2, space="PSUM"))

        ident_t = singles.tile([P, P], F32)
        nc.sync.dma_start(ident_t, ident_d)
        allT = singles.tile([P, 2, B], F8)
        mineT = singles.tile([P, 2, bpc], F8)
        stats_sb = singles.tile([P, 5 * mrow], F32)
        rcps_all = singles.tile([P, nt_all], F32)
        nc.sync.dma_start(rcps_all, rcp_all_d.rearrange("t p -> p t"))
        rcps_mine = singles.tile([P, nt_mine], F32)
        nc.sync.dma_start(rcps_mine, rcp_mine_d.rearrange("t p -> p t"))

        def prologue_chunk(srcs_r, t0, n, rcps, rt0, destT, col0, mine_base):
            """Process a chunk of n row tiles into destT[:, :, col0*P ...]."""
            rawc = raw_pool.tile([P, n, D], F32, tag="raw")
            bi = 0
            while bi < n:
                bn = min(DMA_BATCH, n - bi)
                src = srcs_r[t0 + bi:t0 + bi + bn, :, :].rearrange("n p d -> p n d")
                nc.sync.dma_start(rawc[:, bi:bi + bn, :], src)
                bi += bn
            if mine_base is not None:
                for i in range(n):
                    mi = mine_base + i
                    scr = scr_pool.tile([P, D], F32, tag="scr")
                    nc.vector.scalar_tensor_tensor(
                        out=scr, in0=rawc[:, i, :], scalar=1.0,
                        in1=rawc[:, i, :], op0=MUL, op1=MUL,
                        accum_out=stats_sb[:, mi:mi + 1])
                    nc.vector.reduce_sum(stats_sb[:, mrow + mi:mrow + mi + 1],
                                         rawc[:, i, :], axis=AX)
            # one fused scale+cast: bf16 = raw * rcp_row (stride-0 bcast)
            b16 = b16_pool.tile([P, n, D], BF16, tag="b16")
            rcp_b = rcps[:, rt0:rt0 + n, None].broadcast_to([P, n, D])
            nc.vector.scalar_tensor_tensor(
                out=b16, in0=rawc, scalar=1.0, in1=rcp_b, op0=MUL, op1=MUL)
            # restack to DRAM rows, XBAR-transpose back, cast to fp8
            stage = stage_pool.tile([n * P, D], BF16, tag="stage")
            nc.sync.dma_start(
                stage.rearrange("(t p) d -> p t d", p=P), b16)
            c16 = c16_pool.tile([P, 2, n * P], BF16, tag="c16")
            nc.sync.dma_start_transpose(c16, stage[:, :])
            nc.vector.tensor_copy(
                out=destT[:, :, col0 * P:col0 * P + n * P], in_=c16)

        # ---- own rows first (mineT + per-row stats) ----
        prologue_chunk(fm_r, 0, nt_mine, rcps_mine, 0, mineT, 0, 0)

        # ---- exact diagonal terms ----
        # psd reproduces, bit-for-bit, the diagonal elements the big matmul
        # produces (same DoubleRow datapath, same 512-col window offset);
        # exp through the same ACT path then a masked row-reduce against
        # the identity extracts e_ii, so the host-side pos_sum = S_same - d
        # subtraction cancels exactly.
        W = 512
        for m in range(mrow):
            psd = ps_pool.tile([P, gw], F32, tag="ps", name=f"psd{m}")
            c0 = (m * P // W) * W              # W-col group holding block m
            off = m * P - c0                   # block-local diag offset
            nc.tensor.matmul(psd[:, 0:W], mineT[:, :, m * P:(m + 1) * P],
                             mineT[:, :, c0:c0 + W], start=True, stop=True,
                             perf_mode=DR)
            nc.scalar.activation(psd[:, off:off + P], psd[:, off:off + P],
                                 AF.Exp, scale=EXP_SCALE)
            scrd = scr_pool.tile([P, D], F32, tag="scr", name=f"scrd{m}")
            nc.vector.scalar_tensor_tensor(
                out=scrd[:, 0:P], in0=psd[:, off:off + P], scalar=1.0,
                in1=ident_t, op0=MUL, op1=MUL,
                accum_out=stats_sb[:, 4 * mrow + m:4 * mrow + m + 1])

        accs = [acc_pool.tile([P, nslots], F32, tag="acc", name=f"acc{m}")
                for m in range(mrow)]

        # ---- interleaved: column-chunk prologue + that group's matmuls ----
        tiles_per_g = gw // P
        for g in range(ng):
            t0 = g * tiles_per_g
            prologue_chunk(fa_r, t0, tiles_per_g, rcps_all, t0, allT, t0, None)
            for m in range(mrow):
                psg = ps_pool.tile([P, gw], F32, tag="ps")
                lhsT = mineT[:, :, m * P:(m + 1) * P]
                for sub in range(nsub):
                    ncol = (g * nsub + sub) * 512
                    nc.tensor.matmul(
                        psg[:, sub * 512:(sub + 1) * 512], lhsT,
                        allT[:, :, ncol:ncol + 512],
                        start=True, stop=True, perf_mode=DR)
                for slot, (gg, s, e, _lab) in enumerate(ranges):
                    if gg != g:
                        continue
                    rs, re = s - g * gw, e - g * gw
                    nc.scalar.activation(
                        psg[:, rs:re], psg[:, rs:re], AF.Exp,
                        scale=EXP_SCALE,
                        accum_out=accs[m][:, slot:slot + 1])

        # ---- per-row-tile S0/S1 ----
        for m in range(mrow):
            s0 = stats_sb[:, 2 * mrow + m:2 * mrow + m + 1]
            s1 = stats_sb[:, 3 * mrow + m:3 * mrow + m + 1]
            if k0 > 0:
                nc.vector.reduce_sum(s0, accs[m][:, 0:k0], axis=AX)
            else:
                nc.vector.memset(s0, 0.0)
            if k1 > 0:
                nc.vector.reduce_sum(s1, accs[m][:, k0:nslots], axis=AX)
            else:
                nc.vector.memset(s1, 0.0)

        nc.sync.dma_start(stats_d, stats_sb)

    nc.compile()
    return nc


_PROGRAM_CACHE = {}


def _get_program(n0):
    key = (n0, DMA_BATCH)
    if key not in _PROGRAM_CACHE:
        _PROGRAM_CACHE[key] = build_program(n0)
    return _PROGRAM_CACHE[key]


def run_device(features, labels, trace=False):
    """Run the Bass kernel on 8 cores.  Returns (per-row device stats dict
    aligned to the label-sorted permutation, permutation order, n0, raw
    BassKernelResults)."""
    B, d = features.shape
    assert d == D and B % NCORES == 0
    bpc = B // NCORES
    mrow = bpc // P

    order = np.argsort(labels, kind="stable")
    n0 = int((labels == 0).sum())
    fp = np.ascontiguousarray(features[order]).astype(np.float32, copy=False)

    # host-side normalization helper: rcp = 16/|f| (fp32; only consistency
    # matters -- the same value scales a row wherever it appears)
    rn2 = np.einsum("ij,ij->i", fp.astype(np.float64), fp.astype(np.float64))
    rcp = (FSCALE / np.sqrt(np.maximum(rn2, 1e-24))).astype(np.float32)
    rcp_all = np.ascontiguousarray(rcp.reshape(B // P, P))

    nc = _get_program(n0)
    ident = np.eye(P, dtype=np.float32)
    in_maps = [
        {"feat_all": fp,
         "feat_mine": np.ascontiguousarray(fp[c * bpc:(c + 1) * bpc]),
         "rcp_all": rcp_all,
         "rcp_mine": np.ascontiguousarray(
             rcp_all[c * mrow:(c + 1) * mrow]),
         "ident": ident}
        for c in range(NCORES)
    ]
    res = run_bass_kernel_spmd(nc, in_maps, list(range(NCORES)), trace=trace)

    parts = []
    for c in range(NCORES):
        st = res.results[c]["stats"]          # [128, 5*mrow]
        arr = st.reshape(P, 5, mrow).transpose(1, 2, 0).reshape(5, bpc)
        parts.append(arr)
    full = np.concatenate(parts, axis=1)      # [5, B] in permuted row order
    stats = {"norms2": full[0], "rowsum": full[1], "S0": full[2],
             "S1": full[3], "d": full[4]}
    return stats, order, n0, res


def finalize(stats, order, n0, labels, normal_center, running_sigma, B):
    """Host O(B) finalization mirroring the reference formulas (float64)."""
    labels_p = labels[order]
    nmf = (labels_p == 0)
    amf = (labels_p == 1)
    norms2 = stats["norms2"].astype(np.float64)
    rowsum = stats["rowsum"].astype(np.float64)
    S0 = stats["S0"].astype(np.float64)
    S1 = stats["S1"].astype(np.float64)
    ddiag = stats["d"].astype(np.float64)

    dist_sq = norms2  # center == 0
    n_normal = float(nmf.sum())

    with np.errstate(divide="ignore", invalid="ignore"):
        n_el = n_normal * D
        masked_sum = float((rowsum * nmf).sum())
        mean = masked_sum / n_el
        sum_sq_m = float((norms2 * nmf).sum())
        var = (sum_sq_m - 2.0 * mean * masked_sum + mean * mean * n_el) / (n_el - 1.0)
        sigma_new = 0.9 * float(running_sigma) + 0.1 * np.sqrt(var)

        m_adaptive = (MARGIN_BASE + LAMBDA_SIGMA * sigma_new
                      + LAMBDA_RESOLUTION * (1.0 - RESOLUTION_RATIO))
        dist = np.sqrt(np.maximum(dist_sq, 0.0))
        r_center = dist_sq * nmf
        r_margin = np.maximum(m_adaptive - dist, 0.0) * amf

        S_same = np.where(nmf, S0, S1)
        S_diff = np.where(nmf, S1, S0)
        pos_sum = S_same - ddiag
        neg_sum = S_diff
        n1 = B - n0
        cnt_pos = np.where(nmf, n0 - 1, n1 - 1)
        cnt_neg = np.where(nmf, n1, n0)
        has_both = (cnt_pos > 0) & (cnt_neg > 0)
        pos_safe = np.where(has_both, np.maximum(pos_sum, 1e-12), 1.0)
        den_safe = np.where(has_both, pos_sum + neg_sum + 1e-8, 1.0)
        r_con = np.where(has_both, -np.log(pos_safe / den_safe), 0.0)

        raw_total = ALPHA * r_center + BETA * r_margin + GAMMA * r_con
        total = raw_total.mean()
    return np.array(total, dtype=np.float32)


def _finalize_general_center(stats, order, n0, labels, normal_center,
                             running_sigma, B, features):
    """Fallback for a nonzero normal_center (not used for spec inputs)."""
    labels_p = labels[order]
    fp = features[order].astype(np.float64)
    c = np.asarray(normal_center, dtype=np.float64)
    qc = fp @ c
    norms2 = stats["norms2"].astype(np.float64)
    dist_sq = norms2 - 2.0 * qc + float((c * c).sum())
    nmf = (labels_p == 0)
    amf = (labels_p == 1)
    rowsum = stats["rowsum"].astype(np.float64)
    S0 = stats["S0"].astype(np.float64)
    S1 = stats["S1"].astype(np.float64)
    ddiag = stats["d"].astype(np.float64)
    n_normal = float(nmf.sum())
    with np.errstate(divide="ignore", invalid="ignore"):
        n_el = n_normal * D
        masked_sum = float((rowsum * nmf).sum())
        mean = masked_sum / n_el
        sum_sq_m = float((norms2 * nmf).sum())
        var = (sum_sq_m - 2.0 * mean * masked_sum + mean * mean * n_el) / (n_el - 1.0)
        sigma_new = 0.9 * float(running_sigma) + 0.1 * np.sqrt(var)
        m_adaptive = (MARGIN_BASE + LAMBDA_SIGMA * sigma_new
                      + LAMBDA_RESOLUTION * (1.0 - RESOLUTION_RATIO))
        dist = np.sqrt(np.maximum(dist_sq, 0.0))
        r_center = dist_sq * nmf
        r_margin = np.maximum(m_adaptive - dist, 0.0) * amf
        S_same = np.where(nmf, S0, S1)
        S_diff = np.where(nmf, S1, S0)
        pos_sum = S_same - ddiag
        neg_sum = S_diff
        n1 = B - n0
        cnt_pos = np.where(nmf, n0 - 1, n1 - 1)
        cnt_neg = np.where(nmf, n1, n0)
        has_both = (cnt_pos > 0) & (cnt_neg > 0)
        pos_safe = np.where(has_both, np.maximum(pos_sum, 1e-12), 1.0)
        den_safe = np.where(has_both, pos_sum + neg_sum + 1e-8, 1.0)
        r_con = np.where(has_both, -np.log(pos_safe / den_safe), 0.0)
        total = (ALPHA * r_center + BETA * r_margin + GAMMA * r_con).mean()
    return np.array(total, dtype=np.float32)


def kernel(features, labels, normal_center, running_sigma):
    features = np.asarray(features, dtype=np.float32)
    labels = np.asarray(labels, dtype=np.int32)
    normal_center = np.asarray(normal_center, dtype=np.float32)
    running_sigma = np.float32(np.asarray(running_sigma))
    B = features.shape[0]

    stats, order, n0, _res = run_device(features, labels)
    if float((np.asarray(normal_center, np.float64) ** 2).sum()) != 0.0:
        return _finalize_general_center(stats, order, n0, labels,
                                        normal_center, running_sigma, B,
                                        features)
    return finalize(stats, order, n0, labels, normal_center, running_sigma, B)
